# revision 1
# baseline (speedup 1.0000x reference)
"""GraphSAGE-style mean-aggregator encoder on Trainium2, 8-core SPMD.

Computation (per the reference):
    neigh = features[neigh_idx].mean(1)         # [B, F]
    self_ = features[nodes]                     # [B, F]
    out   = relu(W @ concat(self_, neigh).T)    # [E, B]

Sharding: data-parallel over the node batch B=16384 -> 2048 nodes/core.
features + (pre-transposed, pre-scaled) weight replicated per core.

Per-core kernel, per 128-node tile (16 tiles):
  - 11 single-index indirect DMAs (one per sample) gather 128 rows each
    into whole [128, 512] tiles. HW constraints found empirically: the
    multi-index offset-AP form is mis-ordered and ~70x slower, the offset
    AP must start at a tile base, and indirect-DMA writes to nonzero SBUF
    offsets fault the exec unit. Gathers pipeline at ~1.4us per 256KB
    (Q7 SWDGE descriptor-emission bound, ~181 GB/s/core).
  - neighbor mean via chained DVE adds (1/10 pre-folded into W's
    neighbor half)
  - PE transposes the 8 [128,128] chunks of [self | neigh_sum] via
    identity matmuls, ACT copies PSUM->SBUF; groups of 4 tiles pack rhs
    to N=512 so each W-chunk LoadStationary amortizes (fp32 PE is 4-pass)
  - ACT relu PSUM -> a persistent [128, 2048] output buffer; single 1MB
    store at the end.

Measured on 8xTRN2 (NTFF profile): 282.6us, rel err 4.3e-07.
"""

import numpy as np
from contextlib import ExitStack

import concourse.bass as bass
import concourse.mybir as mybir
import concourse.tile as tile
from concourse import bacc
from concourse.bass_utils import run_bass_kernel_spmd

NCORES = 8
B = 16384
BC = B // NCORES  # 2048 nodes per core
S = 10            # neighbor samples
J = S + 1         # gathered rows per node (self + neighbors)
F = 512           # feature dim
E = 128           # embed dim
NNODES = 200000
P = 128
TILES = BC // P   # 16
IDXW = 16         # padded width of the packed index rows

_CACHE = {}


def build_nc():
    nc = bacc.Bacc(
        "TRN2",
        target_bir_lowering=False,
        debug=False,
        num_devices=NCORES,
    )

    gidx = nc.dram_tensor("gidx", [BC, IDXW], mybir.dt.int32, kind="ExternalInput").ap()
    features = nc.dram_tensor(
        "features", [NNODES, F], mybir.dt.float32, kind="ExternalInput"
    ).ap()
    # host-preprocessed: W^T with the neighbor half pre-scaled by 1/S -> [2F, E]
    wt = nc.dram_tensor("wt", [2 * F, E], mybir.dt.float32, kind="ExternalInput").ap()
    ident = nc.dram_tensor("ident", [P, P], mybir.dt.float32, kind="ExternalInput").ap()
    out = nc.dram_tensor("out", [E, BC], mybir.dt.float32, kind="ExternalOutput").ap()

    KCHUNKS = 2 * F // P  # 8

    with tile.TileContext(nc) as tc, ExitStack() as ctx:
        consts = ctx.enter_context(tc.tile_pool(name="consts", bufs=1))
        stpool = ctx.enter_context(tc.tile_pool(name="stpool", bufs=1))
        gpool = ctx.enter_context(tc.tile_pool(name="gpool", bufs=4))
        spool = ctx.enter_context(tc.tile_pool(name="spool", bufs=3))
        ctpool = ctx.enter_context(tc.tile_pool(name="ctpool", bufs=12))
        psum_t = ctx.enter_context(tc.tile_pool(name="psum_t", bufs=4, space="PSUM"))
        psum_o = ctx.enter_context(tc.tile_pool(name="psum_o", bufs=2, space="PSUM"))

        # indices first: the staging copies (and thus the gather pipeline)
        # depend on them
        idx_all = consts.tile([P, TILES * IDXW], mybir.dt.int32)
        nc.sync.dma_start(
            out=idx_all[:].rearrange("p (t w) -> p t w", t=TILES),
            in_=gidx.rearrange("(t p) w -> p t w", t=TILES),
        )

        identity = consts.tile([P, P], mybir.dt.float32)
        nc.sync.dma_start(out=identity[:], in_=ident[:])

        # W^T chunks: wt dram rows (k p) -> sbuf [p, (k e)]
        wt_sb = consts.tile([P, KCHUNKS * E], mybir.dt.float32)
        nc.sync.dma_start(
            out=wt_sb[:].rearrange("p (k e) -> p k e", k=KCHUNKS),
            in_=wt.rearrange("(k p) e -> p k e", k=KCHUNKS),
        )

        out_sb = consts.tile([E, BC], mybir.dt.float32)

        # Prologue: stage every index column into its own contiguous [P,1]
        # tile. The HW descriptor generator only reads the offset AP
        # correctly when it starts at the tile base (offset 0), and doing
        # all copies up front keeps them off the per-tile critical path
        # (the DVE queue is in-order; interleaving them with the reduces
        # would stall the gather pipeline).
        stages = []
        iview = idx_all[:].rearrange("p (t w) -> p t w", t=TILES)
        for t in range(TILES):
            row = []
            for j in range(J):
                st = stpool.tile(
                    [P, 1], mybir.dt.int32, tag=f"st{t}_{j}", name=f"st{t}_{j}"
                )
                nc.vector.tensor_copy(out=st[:], in_=iview[:, t, j : j + 1])
                row.append(st)
            stages.append(row)

        # Process tiles in groups of 4: the transposed chunks of 4 tiles are
        # packed into [P, 512] rhs tiles so each W-chunk LoadStationary is
        # amortized over N=512 (fp32 matmuls are 4-pass; halving PE overhead
        # keeps it off the critical path).
        GRP = 4
        for gi in range(TILES // GRP):
            cts = [
                ctpool.tile(
                    [P, GRP * P], mybir.dt.float32, tag=f"ct{k}", bufs=2,
                    name=f"ct{gi}_{k}",
                )
                for k in range(KCHUNKS)
            ]
            for ti in range(GRP):
                t = gi * GRP + ti
                # one single-index gather per sample into its own whole tile:
                # the multi-index form is mis-ordered and pathologically slow
                # on HW, and indirect-DMA writes to nonzero SBUF offsets fault
                # the exec unit — every gather dest must be a tile base.
                gs = []
                for j in range(J):
                    gj = gpool.tile(
                        [P, F], mybir.dt.float32, tag=f"g{j}", bufs=3,
                        name=f"g{t}_{j}",
                    )
                    nc.gpsimd.indirect_dma_start(
                        out=gj[:],
                        out_offset=None,
                        in_=features[:],
                        in_offset=bass.IndirectOffsetOnAxis(
                            ap=stages[t][j][:], axis=0
                        ),
                    )
                    gs.append(gj)

                # neighbor sum: chained adds
                nsum = spool.tile([P, F], mybir.dt.float32)
                nc.vector.tensor_add(out=nsum[:], in0=gs[1][:], in1=gs[2][:])
                for j in range(3, J):
                    nc.vector.tensor_add(out=nsum[:], in0=nsum[:], in1=gs[j][:])

                for k in range(KCHUNKS):
                    if k < 4:
                        src = gs[0][:, k * P : (k + 1) * P]
                    else:
                        src = nsum[:, (k - 4) * P : (k - 3) * P]
                    pt = psum_t.tile([P, P], mybir.dt.float32)
                    nc.tensor.transpose(out=pt[:], in_=src, identity=identity[:])
                    nc.scalar.copy(out=cts[k][:, ti * P : (ti + 1) * P], in_=pt[:])

            po = psum_o.tile([E, GRP * P], mybir.dt.float32)
            for k in range(KCHUNKS):
                nc.tensor.matmul(
                    out=po[:],
                    lhsT=wt_sb[:, k * E : (k + 1) * E],
                    rhs=cts[k][:],
                    start=(k == 0),
                    stop=(k == KCHUNKS - 1),
                )

            nc.scalar.activation(
                out=out_sb[:, gi * GRP * P : (gi + 1) * GRP * P],
                in_=po[:],
                func=mybir.ActivationFunctionType.Relu,
            )

        nc.sync.dma_start(out=out[:], in_=out_sb[:])

    nc.compile()
    return nc


def _get_nc():
    if "nc" not in _CACHE:
        _CACHE["nc"] = build_nc()
    return _CACHE["nc"]


def make_in_maps(nodes, neigh_idx, features, weight):
    nodes = np.asarray(nodes, dtype=np.int32)
    neigh_idx = np.asarray(neigh_idx, dtype=np.int32)
    features = np.ascontiguousarray(np.asarray(features, dtype=np.float32))
    weight = np.asarray(weight, dtype=np.float32)

    gidx = np.zeros((B, IDXW), dtype=np.int32)
    gidx[:, 0] = nodes
    gidx[:, 1 : J] = neigh_idx

    w = weight.copy()
    w[:, F:] *= 1.0 / S
    wt = np.ascontiguousarray(w.T)  # [2F, E]
    ident = np.eye(P, dtype=np.float32)

    return [
        {
            "gidx": np.ascontiguousarray(gidx[c * BC : (c + 1) * BC]),
            "features": features,
            "wt": wt,
            "ident": ident,
        }
        for c in range(NCORES)
    ]


def run(nodes, neigh_idx, features, weight, trace=False):
    nc = _get_nc()
    in_maps = make_in_maps(nodes, neigh_idx, features, weight)
    res = run_bass_kernel_spmd(nc, in_maps, list(range(NCORES)), trace=trace)
    full = np.concatenate([res.results[c]["out"] for c in range(NCORES)], axis=1)
    return full, res


def kernel(nodes, neigh_idx, features, weight):
    full, _ = run(nodes, neigh_idx, features, weight, trace=False)
    return full

